# revision 7
# baseline (speedup 1.0000x reference)
"""Trainium2 Bass kernel for GroupNorm + multi-head self-attention block.

Reference computation (per batch element):
    xn  = GroupNorm(x; 32 groups, eps=1e-5) * norm_w + norm_b
    qkv = qkv_w @ xn + qkv_b          (1x1 conv == channel matmul)
    q,k,v split; 4 heads of dh=128 over 1024 spatial positions
    attn = softmax(q^T k * C**-0.5); out = attn @ v
    out = proj_w @ out + proj_b + xn

Sharding: pure data-parallel over batch (16 batches / 8 cores = 2 per core),
no collectives. All matmuls run in float32r (FP22 truncated fp32, single PE
pass) with fp32 PSUM accumulation.

Layouts (per core):
    xn      4 c-tiles [128, BPC, 1024]   channels on partitions
    q,k     per b [128, 4(head), 1024]   head channels on partitions
    v_T     per b [128, 8(jt), 512]      key position j%128 on partitions
    S_T     PSUM [128(j), 512(i)]        scores transposed
    attnout per b [128, 4(head=ctile), 1024(i)]  ready as proj rhs
"""

from contextlib import ExitStack

import numpy as np

B = 16          # full batch
C = 512         # channels
S = 1024        # spatial (32*32)
HEADS = 4
DH = C // HEADS         # 128, head dim == partition tile
GROUPS = 32
EPS = 1e-5
NCORES = 8
BPC = B // NCORES       # 2 batches per core
CT = C // 128           # 4 channel tiles
SCALE = float(C) ** -0.5
JT = S // 128           # 8 j-tiles (key positions)
NH = S // 512           # 2 free-dim halves

_CACHE = {}


def _emit(tc, io):
    from concourse import mybir

    nc = tc.nc
    f32 = mybir.dt.float32
    f32r = mybir.dt.float32r
    Act = mybir.ActivationFunctionType
    Alu = mybir.AluOpType

    x_d = io["x"]
    out_d = io["out"]

    with ExitStack() as ctx:
        consts = ctx.enter_context(tc.tile_pool(name="consts", bufs=1))
        xn_pool = ctx.enter_context(tc.tile_pool(name="xn_pool", bufs=1))
        stats = ctx.enter_context(tc.tile_pool(name="stats", bufs=4))
        qk_pool = ctx.enter_context(tc.tile_pool(name="qk_pool", bufs=1))
        vt_pool = ctx.enter_context(tc.tile_pool(name="vt_pool", bufs=1))
        ao_pool = ctx.enter_context(tc.tile_pool(name="ao_pool", bufs=1))
        e_pool = ctx.enter_context(tc.tile_pool(name="e_pool", bufs=3))
        rc_pool = ctx.enter_context(tc.tile_pool(name="rc_pool", bufs=2))
        fo_pool = ctx.enter_context(tc.tile_pool(name="fo_pool", bufs=4))
        # PSUM pools: mm1(2) + s(2) + o(2) + dn(2) = 8 banks
        mm1 = ctx.enter_context(tc.tile_pool(name="mm1", bufs=2, space="PSUM"))
        s_ps = ctx.enter_context(tc.tile_pool(name="s_ps", bufs=2, space="PSUM"))
        o_ps = ctx.enter_context(tc.tile_pool(name="o_ps", bufs=1, space="PSUM"))
        dn_ps = ctx.enter_context(tc.tile_pool(name="dn_ps", bufs=1, space="PSUM"))

        # ---- constants ----
        qkvT_sb = []
        for k in range(CT):
            t = consts.tile([128, 3 * C], f32r, name=f"qkvT{k}")
            nc.sync.dma_start(out=t, in_=io["qkvT"][k * 128:(k + 1) * 128, :])
            qkvT_sb.append(t)
        projT_sb = []
        for k in range(CT):
            t = consts.tile([128, C], f32r, name=f"projT{k}")
            nc.sync.dma_start(out=t, in_=io["projT"][k * 128:(k + 1) * 128, :])
            projT_sb.append(t)
        qkvb_sb = consts.tile([128, 12], f32, name="qkvb_sb")
        nc.sync.dma_start(out=qkvb_sb, in_=io["qkvb"])
        gnw_sb = consts.tile([128, CT], f32, name="gnw_sb")
        nc.sync.dma_start(out=gnw_sb, in_=io["gnw"])
        gnb_sb = consts.tile([128, CT], f32, name="gnb_sb")
        nc.sync.dma_start(out=gnb_sb, in_=io["gnb"])
        projb_sb = consts.tile([128, CT], f32, name="projb_sb")
        nc.sync.dma_start(out=projb_sb, in_=io["projb"])
        ones_sb = consts.tile([128, 128], f32r, name="ones_sb")
        nc.sync.dma_start(out=ones_sb, in_=io["ones"])
        indp_sb = consts.tile([128, 8], f32r, name="indp_sb")
        nc.sync.dma_start(out=indp_sb, in_=io["indp"])
        indb_sb = consts.tile([8, 128], f32r, name="indb_sb")
        nc.sync.dma_start(out=indb_sb, in_=io["indb"])
        eps_sb = consts.tile([8, 1], f32, name="eps_sb")
        nc.vector.memset(eps_sb, EPS)

        # ---- load x (c-tiles, channels on partitions) ----
        xn = []
        for k in range(CT):
            t = xn_pool.tile([128, BPC, S], f32r, name=f"xn{k}")
            nc.sync.dma_start(
                out=t,
                in_=x_d[:, k * 128:(k + 1) * 128, :].rearrange("b c s -> c b s"),
            )
            xn.append(t)

        # ---- GroupNorm ----
        for k in range(CT):
            xt = xn[k]
            # per-channel mean/var/mean^2 for each batch
            sb_stf = stats.tile([128, BPC, 3], f32, name="sb_stf")
            sb_st = stats.tile([128, BPC, 3], f32r, name="sb_st")
            for b in range(BPC):
                bn6 = stats.tile([128, 2, 6], f32, name="bn6")
                for u in range(2):
                    nc.vector.bn_stats(
                        out=bn6[:, u, :], in_=xt[:, b, u * 512:(u + 1) * 512]
                    )
                nc.vector.bn_aggr(out=sb_stf[:, b, 0:2], in_=bn6)
                nc.vector.tensor_mul(
                    sb_stf[:, b, 2:3], sb_stf[:, b, 0:1], sb_stf[:, b, 0:1]
                )
            nc.vector.tensor_copy(out=sb_st, in_=sb_stf)
            # pool over 16-channel groups: pg[g, (b, {mean, var, mean2})] (x 1/16)
            pgt = mm1.tile([128, 512], f32, name="gn_ps", tag="mm")
            pg = pgt[0:8, 0:BPC * 3].rearrange("g (b t) -> g b t", b=BPC)
            nc.tensor.matmul(
                pg,
                lhsT=indp_sb.bitcast(f32r),
                rhs=sb_st.bitcast(f32r),
                start=True,
                stop=True,
            )
            # g_sb cols: [mean_b0, mean_b1, rstd_b0, rstd_b1]
            pgs = stats.tile([8, BPC, 3], f32, name="pgs")
            nc.scalar.copy(out=pgs, in_=pg)
            g_sb = stats.tile([8, 2 * BPC], f32r, name="g_sb")
            tmp8 = stats.tile([8, 2 * BPC], f32, name="tmp8")
            nc.vector.tensor_copy(out=g_sb[:, 0:BPC], in_=pgs[:, :, 0])
            nc.vector.tensor_mul(tmp8[:, 0:BPC], pgs[:, :, 0], pgs[:, :, 0])
            nc.vector.tensor_add(tmp8[:, BPC:], pgs[:, :, 1], pgs[:, :, 2])
            nc.vector.tensor_sub(tmp8[:, BPC:], tmp8[:, BPC:], tmp8[:, 0:BPC])
            # rstd = 1/sqrt(var + eps)
            nc.scalar.activation(
                out=g_sb[:, BPC:], in_=tmp8[:, BPC:], func=Act.Sqrt, bias=eps_sb
            )
            with nc.allow_low_precision("fp22 matmul input rounding"):
                nc.vector.reciprocal(out=g_sb[:, BPC:], in_=g_sb[:, BPC:])
            # broadcast group stats to channels
            bct = mm1.tile([128, 512], f32, name="gn_ps", tag="mm")
            bc = bct[:, 0:2 * BPC]
            nc.tensor.matmul(
                bc,
                lhsT=indb_sb.bitcast(f32r),
                rhs=g_sb.bitcast(f32r),
                start=True,
                stop=True,
            )
            # sc cols: [negbias_b0, negbias_b1, scale_b0, scale_b1]
            sc = stats.tile([128, 2 * BPC], f32, name="sc")
            for b in range(BPC):
                nc.vector.tensor_scalar_mul(
                    sc[:, BPC + b:BPC + b + 1],
                    bc[:, BPC + b:BPC + b + 1],
                    gnw_sb[:, k:k + 1],
                )
                nc.vector.tensor_mul(
                    sc[:, b:b + 1], bc[:, b:b + 1], sc[:, BPC + b:BPC + b + 1]
                )
                nc.vector.tensor_scalar(
                    sc[:, b:b + 1],
                    sc[:, b:b + 1],
                    gnb_sb[:, k:k + 1],
                    None,
                    op0=Alu.subtract,
                )
                # xn = x*scale - negbias  (in place)
                nc.vector.tensor_scalar(
                    xt[:, b, :],
                    xt[:, b, :],
                    sc[:, BPC + b:BPC + b + 1],
                    sc[:, b:b + 1],
                    op0=Alu.mult,
                    op1=Alu.subtract,
                )

        # ---- per-batch: qkv -> attention -> proj ----
        for b in range(BPC):
            # q, k: [128, head, 1024]; m-tile 0..3 -> q head, 4..7 -> k head
            q_sb = qk_pool.tile([128, HEADS, S], f32r, name="q_sb")
            k_sb = qk_pool.tile([128, HEADS, S], f32r, name="k_sb")
            for m in range(2 * HEADS):
                dst = q_sb if m < HEADS else k_sb
                for n in range(NH):
                    ps = mm1.tile([128, 512], f32, name="qk_ps", tag="mm")
                    for kk in range(CT):
                        nc.tensor.matmul(
                            ps,
                            lhsT=qkvT_sb[kk][:, m * 128:(m + 1) * 128].bitcast(f32r),
                            rhs=xn[kk][:, b, n * 512:(n + 1) * 512].bitcast(f32r),
                            start=(kk == 0),
                            stop=(kk == CT - 1),
                        )
                    nc.vector.tensor_scalar_add(
                        dst[:, m % HEADS, n * 512:(n + 1) * 512],
                        ps,
                        qkvb_sb[:, m:m + 1],
                    )
            # v_T: [128(j), jt, 512(cv)]
            vt_sb = vt_pool.tile([128, JT, C], f32r, name="vt_sb")
            for jt in range(JT):
                ps = mm1.tile([128, 512], f32, name="qk_ps", tag="mm")
                for kk in range(CT):
                    nc.tensor.matmul(
                        ps,
                        lhsT=xn[kk][:, b, jt * 128:(jt + 1) * 128].bitcast(f32r),
                        rhs=qkvT_sb[kk][:, 2 * C:3 * C].bitcast(f32r),
                        start=(kk == 0),
                        stop=(kk == CT - 1),
                    )
                nc.scalar.copy(out=vt_sb[:, jt, :], in_=ps)

            # attention per head
            ao_sb = ao_pool.tile([128, HEADS, S], f32r, name="ao_sb")
            for h in range(HEADS):
                dn = dn_ps.tile([1, S], f32, name="dn")
                ot = o_ps.tile([128, S], f32, name="ot")
                for jt in range(JT):
                    et = e_pool.tile([128, S], f32r, name="et")
                    for n in range(NH):
                        sp = s_ps.tile([128, 512], f32, name="sp")
                        nc.tensor.matmul(
                            sp,
                            lhsT=k_sb[:, h, jt * 128:(jt + 1) * 128].bitcast(f32r),
                            rhs=q_sb[:, h, n * 512:(n + 1) * 512].bitcast(f32r),
                            start=True,
                            stop=True,
                        )
                        nc.scalar.activation(
                            out=et[:, n * 512:(n + 1) * 512],
                            in_=sp,
                            func=Act.Exp,
                            scale=SCALE,
                        )
                        nc.tensor.matmul(
                            dn[:, n * 512:(n + 1) * 512],
                            lhsT=ones_sb[:, 0:1].bitcast(f32r),
                            rhs=et[:, n * 512:(n + 1) * 512].bitcast(f32r),
                            start=(jt == 0),
                            stop=(jt == JT - 1),
                        )
                        nc.tensor.matmul(
                            ot[:, n * 512:(n + 1) * 512],
                            lhsT=vt_sb[:, jt, h * 128:(h + 1) * 128].bitcast(f32r),
                            rhs=et[:, n * 512:(n + 1) * 512].bitcast(f32r),
                            start=(jt == 0),
                            stop=(jt == JT - 1),
                        )
                # denominators -> SBUF, broadcast to 128 partitions, reciprocal
                dns = stats.tile([1, S], f32r, name="dns")
                nc.scalar.copy(out=dns, in_=dn)
                rc = rc_pool.tile([128, S], f32, name="rc")
                for n in range(NH):
                    bcp = s_ps.tile([128, 512], f32, name="sp")
                    nc.tensor.matmul(
                        bcp,
                        lhsT=ones_sb[0:1, :].bitcast(f32r),
                        rhs=dns[:, n * 512:(n + 1) * 512].bitcast(f32r),
                        start=True,
                        stop=True,
                    )
                    nc.vector.reciprocal_approx_fast(
                        out=rc[:, n * 512:(n + 1) * 512], in_=bcp
                    )
                # attnout = ot * rc + v_bias
                nc.vector.tensor_mul(ao_sb[:, h, :], ot, rc)
                nc.vector.tensor_scalar_add(
                    ao_sb[:, h, :], ao_sb[:, h, :], qkvb_sb[:, 8 + h:9 + h]
                )

            # proj + residual
            for m in range(CT):
                for n in range(NH):
                    ps = mm1.tile([128, 512], f32, name="qk_ps", tag="mm")
                    for kk in range(CT):
                        nc.tensor.matmul(
                            ps,
                            lhsT=projT_sb[kk][:, m * 128:(m + 1) * 128].bitcast(f32r),
                            rhs=ao_sb[:, kk, n * 512:(n + 1) * 512].bitcast(f32r),
                            start=(kk == 0),
                            stop=(kk == CT - 1),
                        )
                    fo = fo_pool.tile([128, 512], f32, name="fo")
                    # fo = (ps + proj_b) + xn
                    nc.vector.affine_then_add(
                        out=fo,
                        in0=ps,
                        in1=xn[m][:, b, n * 512:(n + 1) * 512],
                        scale=1.0,
                        bias=projb_sb[:, m:m + 1],
                    )
                    nc.sync.dma_start(
                        out=out_d[b, m * 128:(m + 1) * 128, n * 512:(n + 1) * 512],
                        in_=fo,
                    )


def _build_nc():
    import concourse.tile as tile
    from concourse import bacc, mybir

    f32 = mybir.dt.float32
    f32r = mybir.dt.float32r
    nc = bacc.Bacc("TRN2", target_bir_lowering=False, debug=False)
    io = {
        "x": nc.dram_tensor("x", [BPC, C, S], f32r, kind="ExternalInput").ap(),
        "qkvT": nc.dram_tensor("qkvT", [C, 3 * C], f32r, kind="ExternalInput").ap(),
        "projT": nc.dram_tensor("projT", [C, C], f32r, kind="ExternalInput").ap(),
        "qkvb": nc.dram_tensor("qkvb", [128, 12], f32, kind="ExternalInput").ap(),
        "gnw": nc.dram_tensor("gnw", [128, CT], f32, kind="ExternalInput").ap(),
        "gnb": nc.dram_tensor("gnb", [128, CT], f32, kind="ExternalInput").ap(),
        "projb": nc.dram_tensor("projb", [128, CT], f32, kind="ExternalInput").ap(),
        "ones": nc.dram_tensor("ones", [128, 128], f32r, kind="ExternalInput").ap(),
        "indp": nc.dram_tensor("indp", [128, 8], f32r, kind="ExternalInput").ap(),
        "indb": nc.dram_tensor("indb", [8, 128], f32r, kind="ExternalInput").ap(),
        "out": nc.dram_tensor("out", [BPC, C, S], f32, kind="ExternalOutput").ap(),
    }
    with tile.TileContext(nc) as tc:
        _emit(tc, io)
    nc.compile()
    return nc


def get_nc():
    if "nc" not in _CACHE:
        _CACHE["nc"] = _build_nc()
    return _CACHE["nc"]


def make_const_inputs(norm_w, norm_b, qkv_w, qkv_b, proj_w, proj_b):
    """Host-side constant tensors shared by all cores."""
    f = np.float32
    qkvT = np.ascontiguousarray(qkv_w.T, dtype=f)              # [C, 3C]
    projT = np.ascontiguousarray(proj_w.T, dtype=f)            # [C, C]
    qkvb = np.ascontiguousarray(qkv_b.reshape(12, 128).T, dtype=f)
    gnw = np.ascontiguousarray(norm_w.reshape(CT, 128).T, dtype=f)
    gnb = np.ascontiguousarray(norm_b.reshape(CT, 128).T, dtype=f)
    projb = np.ascontiguousarray(proj_b.reshape(CT, 128).T, dtype=f)
    ones = np.ones((128, 128), dtype=f)
    indp = np.zeros((128, 8), dtype=f)
    for p in range(128):
        indp[p, p // 16] = 1.0 / 16.0
    indb = np.zeros((8, 128), dtype=f)
    for p in range(128):
        indb[p // 16, p] = 1.0
    return {
        "qkvT": qkvT, "projT": projT, "qkvb": qkvb,
        "gnw": gnw, "gnb": gnb, "projb": projb,
        "ones": ones, "indp": indp, "indb": indb,
    }


def kernel(x, norm_w, norm_b, qkv_w, qkv_b, proj_w, proj_b, _trace=False):
    from concourse.bass_utils import run_bass_kernel_spmd

    b, c, h, w = x.shape
    assert (b, c, h * w) == (B, C, S), f"unexpected input shape {x.shape}"
    consts = make_const_inputs(norm_w, norm_b, qkv_w, qkv_b, proj_w, proj_b)
    xf = np.ascontiguousarray(x.reshape(B, C, S), dtype=np.float32)
    in_maps = [
        {"x": np.ascontiguousarray(xf[i * BPC:(i + 1) * BPC]), **consts}
        for i in range(NCORES)
    ]
    nc = get_nc()
    res = run_bass_kernel_spmd(
        nc, in_maps, core_ids=list(range(NCORES)), trace=_trace
    )
    out = np.concatenate([r["out"] for r in res.results], axis=0)
    out = out.reshape(B, C, h, w).astype(np.float32)
    if _trace:
        _CACHE["last_results"] = res
    return out


# revision 8
# speedup vs baseline: 1.0189x; 1.0189x over previous
"""Trainium2 Bass kernel for GroupNorm + multi-head self-attention block.

Reference computation (per batch element):
    xn  = GroupNorm(x; 32 groups, eps=1e-5) * norm_w + norm_b
    qkv = qkv_w @ xn + qkv_b          (1x1 conv == channel matmul)
    q,k,v split; 4 heads of dh=128 over 1024 spatial positions
    attn = softmax(q^T k * C**-0.5); out = attn @ v
    out = proj_w @ out + proj_b + xn

Sharding: pure data-parallel over batch (16 batches / 8 cores = 2 per core),
no collectives.

Precision: GroupNorm statistics run in fp32/fp32r; the large matmuls (qkv,
scores, softmax-denominator, attn*v, proj) run in bf16 with fp32 PSUM
accumulation; softmax and all bias/residual arithmetic in fp32.

Layouts (per core):
    xn      4 c-tiles [128, BPC, 1024] f32r   channels on partitions (residual)
    xn_bf   bf16 copy of xn                    matmul operand
    q,k     per b [128, 4(head), 1024] bf16    head channels on partitions
    v_T     per b [128, 8(jt), 512] bf16       key position j%128 on partitions
    S_T     PSUM [128(j), 512(i)] f32          scores transposed
    attnout per b [128, 4(head=ctile), 1024] bf16  ready as proj rhs
"""

from contextlib import ExitStack

import numpy as np

B = 16          # full batch
C = 512         # channels
S = 1024        # spatial (32*32)
HEADS = 4
DH = C // HEADS         # 128, head dim == partition tile
GROUPS = 32
EPS = 1e-5
NCORES = 8
BPC = B // NCORES       # 2 batches per core
CT = C // 128           # 4 channel tiles
SCALE = float(C) ** -0.5
JT = S // 128           # 8 j-tiles (key positions)
NH = S // 512           # 2 free-dim halves

_CACHE = {}


def _emit(tc, io):
    from concourse import mybir

    nc = tc.nc
    f32 = mybir.dt.float32
    f32r = mybir.dt.float32r
    bf16 = mybir.dt.bfloat16
    Act = mybir.ActivationFunctionType
    Alu = mybir.AluOpType

    x_d = io["x"]
    out_d = io["out"]

    with ExitStack() as ctx:
        consts = ctx.enter_context(tc.tile_pool(name="consts", bufs=1))
        xn_pool = ctx.enter_context(tc.tile_pool(name="xn_pool", bufs=1))
        stats = ctx.enter_context(tc.tile_pool(name="stats", bufs=4))
        qk_pool = ctx.enter_context(tc.tile_pool(name="qk_pool", bufs=1))
        vt_pool = ctx.enter_context(tc.tile_pool(name="vt_pool", bufs=1))
        ao_pool = ctx.enter_context(tc.tile_pool(name="ao_pool", bufs=1))
        e_pool = ctx.enter_context(tc.tile_pool(name="e_pool", bufs=3))
        rc_pool = ctx.enter_context(tc.tile_pool(name="rc_pool", bufs=2))
        fo_pool = ctx.enter_context(tc.tile_pool(name="fo_pool", bufs=4))
        # PSUM pools: mm1(2) + s(2) + o(2) + dn(2) = 8 banks
        mm1 = ctx.enter_context(tc.tile_pool(name="mm1", bufs=2, space="PSUM"))
        s_ps = ctx.enter_context(tc.tile_pool(name="s_ps", bufs=2, space="PSUM"))
        o_ps = ctx.enter_context(tc.tile_pool(name="o_ps", bufs=1, space="PSUM"))
        dn_ps = ctx.enter_context(tc.tile_pool(name="dn_ps", bufs=1, space="PSUM"))

        # ---- constants ----
        qkvT_sb = []
        for k in range(CT):
            t = consts.tile([128, 3 * C], bf16, name=f"qkvT{k}")
            nc.sync.dma_start(out=t, in_=io["qkvT"][k * 128:(k + 1) * 128, :])
            qkvT_sb.append(t)
        projT_sb = []
        for k in range(CT):
            t = consts.tile([128, C], bf16, name=f"projT{k}")
            nc.sync.dma_start(out=t, in_=io["projT"][k * 128:(k + 1) * 128, :])
            projT_sb.append(t)
        qkvb_sb = consts.tile([128, 12], f32, name="qkvb_sb")
        nc.sync.dma_start(out=qkvb_sb, in_=io["qkvb"])
        gnw_sb = consts.tile([128, CT], f32, name="gnw_sb")
        nc.sync.dma_start(out=gnw_sb, in_=io["gnw"])
        gnb_sb = consts.tile([128, CT], f32, name="gnb_sb")
        nc.sync.dma_start(out=gnb_sb, in_=io["gnb"])
        projb_sb = consts.tile([128, CT], f32, name="projb_sb")
        nc.sync.dma_start(out=projb_sb, in_=io["projb"])
        indp_sb = consts.tile([128, 8], f32r, name="indp_sb")
        nc.sync.dma_start(out=indp_sb, in_=io["indp"])
        indb_sb = consts.tile([8, 128], f32r, name="indb_sb")
        nc.sync.dma_start(out=indb_sb, in_=io["indb"])
        ones_bf = consts.tile([128, 128], bf16, name="ones_bf")
        nc.vector.memset(ones_bf, 1.0)
        eps_sb = consts.tile([8, 1], f32, name="eps_sb")
        nc.vector.memset(eps_sb, EPS)

        # ---- load x (c-tiles, channels on partitions) ----
        xn = []
        for k in range(CT):
            t = xn_pool.tile([128, BPC, S], f32r, name=f"xn{k}")
            for b in range(BPC):
                nc.sync.dma_start(
                    out=t[:, b, :], in_=x_d[b, k * 128:(k + 1) * 128, :]
                )
            xn.append(t)
        xn_bf = [
            xn_pool.tile([128, BPC, S], bf16, name=f"xnbf{k}") for k in range(CT)
        ]

        # ---- GroupNorm ----
        for k in range(CT):
            xt = xn[k]
            # per-channel mean/var/mean^2 for each batch
            sb_stf = stats.tile([128, BPC, 3], f32, name="sb_stf")
            sb_st = stats.tile([128, BPC, 3], f32r, name="sb_st")
            for b in range(BPC):
                bn6 = stats.tile([128, 2, 6], f32, name="bn6")
                for u in range(2):
                    nc.vector.bn_stats(
                        out=bn6[:, u, :], in_=xt[:, b, u * 512:(u + 1) * 512]
                    )
                nc.vector.bn_aggr(out=sb_stf[:, b, 0:2], in_=bn6)
                nc.vector.tensor_mul(
                    sb_stf[:, b, 2:3], sb_stf[:, b, 0:1], sb_stf[:, b, 0:1]
                )
            nc.vector.tensor_copy(out=sb_st, in_=sb_stf)
            # pool over 16-channel groups: pg[g, (b, {mean, var, mean2})] (x 1/16)
            pgt = mm1.tile([128, 512], f32, name="gn_ps", tag="mm")
            pg = pgt[0:8, 0:BPC * 3].rearrange("g (b t) -> g b t", b=BPC)
            nc.tensor.matmul(pg, lhsT=indp_sb, rhs=sb_st, start=True, stop=True)
            # g_sb cols: [mean_b0, mean_b1, rstd_b0, rstd_b1]
            pgs = stats.tile([8, BPC, 3], f32, name="pgs")
            nc.scalar.copy(out=pgs, in_=pg)
            g_sb = stats.tile([8, 2 * BPC], f32r, name="g_sb")
            tmp8 = stats.tile([8, 2 * BPC], f32, name="tmp8")
            nc.vector.tensor_copy(out=g_sb[:, 0:BPC], in_=pgs[:, :, 0])
            nc.vector.tensor_mul(tmp8[:, 0:BPC], pgs[:, :, 0], pgs[:, :, 0])
            nc.vector.tensor_add(tmp8[:, BPC:], pgs[:, :, 1], pgs[:, :, 2])
            nc.vector.tensor_sub(tmp8[:, BPC:], tmp8[:, BPC:], tmp8[:, 0:BPC])
            # rstd = 1/sqrt(var + eps)
            nc.scalar.activation(
                out=g_sb[:, BPC:], in_=tmp8[:, BPC:], func=Act.Sqrt, bias=eps_sb
            )
            with nc.allow_low_precision("fp22 matmul input rounding"):
                nc.vector.reciprocal(out=g_sb[:, BPC:], in_=g_sb[:, BPC:])
            # broadcast group stats to channels
            bct = mm1.tile([128, 512], f32, name="gn_ps", tag="mm")
            bc = bct[:, 0:2 * BPC]
            nc.tensor.matmul(bc, lhsT=indb_sb, rhs=g_sb, start=True, stop=True)
            # sc cols: [negbias_b0, negbias_b1, scale_b0, scale_b1]
            sc = stats.tile([128, 2 * BPC], f32, name="sc")
            for b in range(BPC):
                nc.vector.tensor_scalar_mul(
                    sc[:, BPC + b:BPC + b + 1],
                    bc[:, BPC + b:BPC + b + 1],
                    gnw_sb[:, k:k + 1],
                )
                nc.vector.tensor_mul(
                    sc[:, b:b + 1], bc[:, b:b + 1], sc[:, BPC + b:BPC + b + 1]
                )
                nc.vector.tensor_scalar(
                    sc[:, b:b + 1],
                    sc[:, b:b + 1],
                    gnb_sb[:, k:k + 1],
                    None,
                    op0=Alu.subtract,
                )
                # xn = x*scale - negbias  (in place), bf16 copy for matmuls
                nc.vector.tensor_scalar(
                    xt[:, b, :],
                    xt[:, b, :],
                    sc[:, BPC + b:BPC + b + 1],
                    sc[:, b:b + 1],
                    op0=Alu.mult,
                    op1=Alu.subtract,
                )
                nc.scalar.copy(out=xn_bf[k][:, b, :], in_=xt[:, b, :])

        # ---- per-batch phases, software-pipelined:
        # qk(0), vT(0), attn(0), qk(1), vT(1), proj(0), attn(1), proj(1)
        q_sb = {}
        k_sb = {}
        vt_sb = {}
        ao_sb = {}

        def emit_qkv(b):
            # q, k: [128, head, 1024]; m-tile 0..3 -> q head, 4..7 -> k head
            q_sb[b] = qk_pool.tile([128, HEADS, S], bf16, name="q_sb")
            k_sb[b] = qk_pool.tile([128, HEADS, S], bf16, name="k_sb")
            for m in range(2 * HEADS):
                dst = q_sb[b] if m < HEADS else k_sb[b]
                for n in range(NH):
                    ps = mm1.tile([128, 512], f32, name="qk_ps", tag="mm")
                    for kk in range(CT):
                        nc.tensor.matmul(
                            ps,
                            lhsT=qkvT_sb[kk][:, m * 128:(m + 1) * 128],
                            rhs=xn_bf[kk][:, b, n * 512:(n + 1) * 512],
                            start=(kk == 0),
                            stop=(kk == CT - 1),
                        )
                    nc.vector.tensor_scalar_add(
                        dst[:, m % HEADS, n * 512:(n + 1) * 512],
                        ps,
                        qkvb_sb[:, m:m + 1],
                    )
            # v_T: [128(j), jt, 512(cv)]
            vt_sb[b] = vt_pool.tile([128, JT, C], bf16, name="vt_sb")
            for jt in range(JT):
                ps = mm1.tile([128, 512], f32, name="qk_ps", tag="mm")
                for kk in range(CT):
                    nc.tensor.matmul(
                        ps,
                        lhsT=xn_bf[kk][:, b, jt * 128:(jt + 1) * 128],
                        rhs=qkvT_sb[kk][:, 2 * C:3 * C],
                        start=(kk == 0),
                        stop=(kk == CT - 1),
                    )
                nc.scalar.copy(out=vt_sb[b][:, jt, :], in_=ps)

        def emit_attn(b):
            ao_sb[b] = ao_pool.tile([128, HEADS, S], bf16, name="ao_sb")
            for h in range(HEADS):
                dn = dn_ps.tile([1, S], f32, name="dn")
                ot = o_ps.tile([128, S], f32, name="ot")
                for jt in range(JT):
                    et = e_pool.tile([128, S], bf16, name="et")
                    for n in range(NH):
                        sp = s_ps.tile([128, 512], f32, name="sp")
                        nc.tensor.matmul(
                            sp,
                            lhsT=k_sb[b][:, h, jt * 128:(jt + 1) * 128],
                            rhs=q_sb[b][:, h, n * 512:(n + 1) * 512],
                            start=True,
                            stop=True,
                        )
                        nc.scalar.activation(
                            out=et[:, n * 512:(n + 1) * 512],
                            in_=sp,
                            func=Act.Exp,
                            scale=SCALE,
                        )
                        nc.tensor.matmul(
                            dn[:, n * 512:(n + 1) * 512],
                            lhsT=ones_bf[:, 0:1],
                            rhs=et[:, n * 512:(n + 1) * 512],
                            start=(jt == 0),
                            stop=(jt == JT - 1),
                        )
                        nc.tensor.matmul(
                            ot[:, n * 512:(n + 1) * 512],
                            lhsT=vt_sb[b][:, jt, h * 128:(h + 1) * 128],
                            rhs=et[:, n * 512:(n + 1) * 512],
                            start=(jt == 0),
                            stop=(jt == JT - 1),
                        )
                # denominators -> SBUF, broadcast to 128 partitions, reciprocal
                dns = stats.tile([1, S], bf16, name="dns")
                nc.scalar.copy(out=dns, in_=dn)
                rc = rc_pool.tile([128, S], f32, name="rc")
                for n in range(NH):
                    bcp = s_ps.tile([128, 512], f32, name="sp")
                    nc.tensor.matmul(
                        bcp,
                        lhsT=ones_bf[0:1, :],
                        rhs=dns[:, n * 512:(n + 1) * 512],
                        start=True,
                        stop=True,
                    )
                    nc.vector.reciprocal_approx_fast(
                        out=rc[:, n * 512:(n + 1) * 512], in_=bcp
                    )
                # attnout = ot * rc + v_bias
                nc.vector.tensor_mul(ao_sb[b][:, h, :], ot, rc)
                nc.vector.tensor_scalar_add(
                    ao_sb[b][:, h, :], ao_sb[b][:, h, :], qkvb_sb[:, 8 + h:9 + h]
                )

        def emit_proj(b):
            for m in range(CT):
                for n in range(NH):
                    ps = mm1.tile([128, 512], f32, name="qk_ps", tag="mm")
                    for kk in range(CT):
                        nc.tensor.matmul(
                            ps,
                            lhsT=projT_sb[kk][:, m * 128:(m + 1) * 128],
                            rhs=ao_sb[b][:, kk, n * 512:(n + 1) * 512],
                            start=(kk == 0),
                            stop=(kk == CT - 1),
                        )
                    fo = fo_pool.tile([128, 512], f32, name="fo")
                    # fo = (ps + proj_b) + xn
                    nc.vector.affine_then_add(
                        out=fo,
                        in0=ps,
                        in1=xn[m][:, b, n * 512:(n + 1) * 512],
                        scale=1.0,
                        bias=projb_sb[:, m:m + 1],
                    )
                    nc.sync.dma_start(
                        out=out_d[b, m * 128:(m + 1) * 128, n * 512:(n + 1) * 512],
                        in_=fo,
                    )

        emit_qkv(0)
        emit_attn(0)
        emit_qkv(1)
        emit_proj(0)
        emit_attn(1)
        emit_proj(1)


def _build_nc():
    import concourse.tile as tile
    from concourse import bacc, mybir

    f32 = mybir.dt.float32
    f32r = mybir.dt.float32r
    bf16 = mybir.dt.bfloat16
    nc = bacc.Bacc("TRN2", target_bir_lowering=False, debug=False)
    io = {
        "x": nc.dram_tensor("x", [BPC, C, S], f32r, kind="ExternalInput").ap(),
        "qkvT": nc.dram_tensor("qkvT", [C, 3 * C], bf16, kind="ExternalInput").ap(),
        "projT": nc.dram_tensor("projT", [C, C], bf16, kind="ExternalInput").ap(),
        "qkvb": nc.dram_tensor("qkvb", [128, 12], f32, kind="ExternalInput").ap(),
        "gnw": nc.dram_tensor("gnw", [128, CT], f32, kind="ExternalInput").ap(),
        "gnb": nc.dram_tensor("gnb", [128, CT], f32, kind="ExternalInput").ap(),
        "projb": nc.dram_tensor("projb", [128, CT], f32, kind="ExternalInput").ap(),
        "indp": nc.dram_tensor("indp", [128, 8], f32r, kind="ExternalInput").ap(),
        "indb": nc.dram_tensor("indb", [8, 128], f32r, kind="ExternalInput").ap(),
        "out": nc.dram_tensor("out", [BPC, C, S], f32, kind="ExternalOutput").ap(),
    }
    with tile.TileContext(nc) as tc:
        _emit(tc, io)
    nc.compile()
    return nc


def get_nc():
    if "nc" not in _CACHE:
        _CACHE["nc"] = _build_nc()
    return _CACHE["nc"]


def make_const_inputs(norm_w, norm_b, qkv_w, qkv_b, proj_w, proj_b):
    """Host-side constant tensors shared by all cores."""
    import ml_dtypes

    f = np.float32
    bf = ml_dtypes.bfloat16
    qkvT = np.ascontiguousarray(qkv_w.T.astype(bf))            # [C, 3C]
    projT = np.ascontiguousarray(proj_w.T.astype(bf))          # [C, C]
    qkvb = np.ascontiguousarray(qkv_b.reshape(12, 128).T, dtype=f)
    gnw = np.ascontiguousarray(norm_w.reshape(CT, 128).T, dtype=f)
    gnb = np.ascontiguousarray(norm_b.reshape(CT, 128).T, dtype=f)
    projb = np.ascontiguousarray(proj_b.reshape(CT, 128).T, dtype=f)
    indp = np.zeros((128, 8), dtype=f)
    for p in range(128):
        indp[p, p // 16] = 1.0 / 16.0
    indb = np.zeros((8, 128), dtype=f)
    for p in range(128):
        indb[p // 16, p] = 1.0
    return {
        "qkvT": qkvT, "projT": projT, "qkvb": qkvb,
        "gnw": gnw, "gnb": gnb, "projb": projb,
        "indp": indp, "indb": indb,
    }


def kernel(x, norm_w, norm_b, qkv_w, qkv_b, proj_w, proj_b, _trace=False):
    from concourse.bass_utils import run_bass_kernel_spmd

    b, c, h, w = x.shape
    assert (b, c, h * w) == (B, C, S), f"unexpected input shape {x.shape}"
    consts = make_const_inputs(norm_w, norm_b, qkv_w, qkv_b, proj_w, proj_b)
    xf = np.ascontiguousarray(x.reshape(B, C, S), dtype=np.float32)
    in_maps = [
        {"x": np.ascontiguousarray(xf[i * BPC:(i + 1) * BPC]), **consts}
        for i in range(NCORES)
    ]
    nc = get_nc()
    res = run_bass_kernel_spmd(
        nc, in_maps, core_ids=list(range(NCORES)), trace=_trace
    )
    out = np.concatenate([r["out"] for r in res.results], axis=0)
    out = out.reshape(B, C, h, w).astype(np.float32)
    if _trace:
        _CACHE["last_results"] = res
    return out


# revision 10
# speedup vs baseline: 1.0878x; 1.0676x over previous
"""Trainium2 Bass kernel for GroupNorm + multi-head self-attention block.

Reference computation (per batch element):
    xn  = GroupNorm(x; 32 groups, eps=1e-5) * norm_w + norm_b
    qkv = qkv_w @ xn + qkv_b          (1x1 conv == channel matmul)
    q,k,v split; 4 heads of dh=128 over 1024 spatial positions
    attn = softmax(q^T k * C**-0.5); out = attn @ v
    out = proj_w @ out + proj_b + xn

Sharding: pure data-parallel over batch (16 batches / 8 cores = 2 per core),
no collectives.

Precision: GroupNorm statistics in fp32 (cross-partition pooling via small
fp32r indicator matmuls); the large matmuls (qkv, scores, softmax
denominator, attn*v, proj) in bf16 with fp32 PSUM accumulation; softmax and
bias/residual arithmetic in fp32.

Schedule highlights:
  - GroupNorm runs per batch so batch 0's qkv matmuls start while batch 1's
    stats are still on the Vector engine.
  - Attention is software-pipelined: denominator/output matmuls trail the
    score matmuls by one j-tile so ScalarE exp latency stays off the PE
    critical path.
  - softmax reciprocal broadcast runs on GpSimd (partition_broadcast).
"""

from contextlib import ExitStack

import numpy as np

B = 16          # full batch
C = 512         # channels
S = 1024        # spatial (32*32)
HEADS = 4
DH = C // HEADS         # 128, head dim == partition tile
GROUPS = 32
EPS = 1e-5
NCORES = 8
BPC = B // NCORES       # 2 batches per core
CT = C // 128           # 4 channel tiles
SCALE = float(C) ** -0.5
JT = S // 128           # 8 j-tiles (key positions)
NH = S // 512           # 2 free-dim halves

_CACHE = {}


def _emit(tc, io):
    from concourse import mybir

    nc = tc.nc
    f32 = mybir.dt.float32
    f32r = mybir.dt.float32r
    bf16 = mybir.dt.bfloat16
    Act = mybir.ActivationFunctionType
    Alu = mybir.AluOpType

    x_d = io["x"]
    out_d = io["out"]

    with ExitStack() as ctx:
        consts = ctx.enter_context(tc.tile_pool(name="consts", bufs=1))
        x_pool = ctx.enter_context(tc.tile_pool(name="x_pool", bufs=3))
        xnbf_pool = ctx.enter_context(tc.tile_pool(name="xnbf_pool", bufs=1))
        stats = ctx.enter_context(tc.tile_pool(name="stats", bufs=4))
        qk_pool = ctx.enter_context(tc.tile_pool(name="qk_pool", bufs=2))
        vt_pool = ctx.enter_context(tc.tile_pool(name="vt_pool", bufs=2))
        ao_pool = ctx.enter_context(tc.tile_pool(name="ao_pool", bufs=2))
        e_pool = ctx.enter_context(tc.tile_pool(name="e_pool", bufs=4))
        rc_pool = ctx.enter_context(tc.tile_pool(name="rc_pool", bufs=2))
        dcb_pool = ctx.enter_context(tc.tile_pool(name="dcb_pool", bufs=2))
        fo_pool = ctx.enter_context(tc.tile_pool(name="fo_pool", bufs=4))
        # PSUM pools: mm1(2) + s(2) + o(2) + dn(2) = 8 banks
        mm1 = ctx.enter_context(tc.tile_pool(name="mm1", bufs=2, space="PSUM"))
        s_ps = ctx.enter_context(tc.tile_pool(name="s_ps", bufs=2, space="PSUM"))
        o_ps = ctx.enter_context(tc.tile_pool(name="o_ps", bufs=1, space="PSUM"))
        dn_ps = ctx.enter_context(tc.tile_pool(name="dn_ps", bufs=1, space="PSUM"))

        # ---- constants ----
        qkvT_sb = []
        for k in range(CT):
            t = consts.tile([128, 3 * C], bf16, name=f"qkvT{k}")
            nc.sync.dma_start(out=t, in_=io["qkvT"][k * 128:(k + 1) * 128, :])
            qkvT_sb.append(t)
        projT_sb = []
        for k in range(CT):
            t = consts.tile([128, C], bf16, name=f"projT{k}")
            nc.sync.dma_start(out=t, in_=io["projT"][k * 128:(k + 1) * 128, :])
            projT_sb.append(t)
        qkvb_sb = consts.tile([128, 12], f32, name="qkvb_sb")
        nc.sync.dma_start(out=qkvb_sb, in_=io["qkvb"])
        gnw_sb = consts.tile([128, CT], f32, name="gnw_sb")
        nc.sync.dma_start(out=gnw_sb, in_=io["gnw"])
        gnb_sb = consts.tile([128, CT], f32, name="gnb_sb")
        nc.sync.dma_start(out=gnb_sb, in_=io["gnb"])
        projb_sb = consts.tile([128, CT], f32, name="projb_sb")
        nc.sync.dma_start(out=projb_sb, in_=io["projb"])
        indp_sb = consts.tile([128, 8], f32r, name="indp_sb")
        nc.sync.dma_start(out=indp_sb, in_=io["indp"])
        indb_sb = consts.tile([8, 128], f32r, name="indb_sb")
        nc.sync.dma_start(out=indb_sb, in_=io["indb"])
        ones_bf = consts.tile([128, 128], bf16, name="ones_bf")
        nc.vector.memset(ones_bf, 1.0)
        eps_sb = consts.tile([8, 1], f32, name="eps_sb")
        nc.vector.memset(eps_sb, EPS)

        # normalized x in bf16, per batch: [128, 1024] per (ctile, b)
        xn_bf = [
            xnbf_pool.tile([128, BPC, S], bf16, name=f"xnbf{k}") for k in range(CT)
        ]

        def emit_gn(b):
            """GroupNorm for batch b -> xn_bf[k][:, b, :]."""
            for k in range(CT):
                xt = x_pool.tile([128, S], f32, name="xt")
                nc.sync.dma_start(out=xt, in_=x_d[b, k * 128:(k + 1) * 128, :])
                # per-channel mean / var / mean^2
                sb_stf = stats.tile([128, 4], f32, name="sb_stf")
                sb_st = stats.tile([128, 4], f32r, name="sb_st")
                bn6 = stats.tile([128, 2, 6], f32, name="bn6")
                for u in range(2):
                    nc.vector.bn_stats(
                        out=bn6[:, u, :], in_=xt[:, u * 512:(u + 1) * 512]
                    )
                nc.vector.bn_aggr(out=sb_stf[:, 0:2], in_=bn6)
                nc.vector.tensor_mul(sb_stf[:, 2:3], sb_stf[:, 0:1], sb_stf[:, 0:1])
                nc.vector.tensor_copy(out=sb_stf[:, 3:4], in_=sb_stf[:, 0:1])
                nc.vector.tensor_copy(out=sb_st, in_=sb_stf)
                # pool over 16-channel groups (x 1/16): pg[g, {mean, var, mean2, pad}]
                pgt = mm1.tile([128, 512], f32, name="gn_ps", tag="mm")
                pg = pgt[0:8, 0:4]
                nc.tensor.matmul(pg, lhsT=indp_sb, rhs=sb_st, start=True, stop=True)
                pgs = stats.tile([8, 4], f32, name="pgs")
                nc.scalar.copy(out=pgs, in_=pg)
                # g_sb cols: [mean_g, rstd_g]
                g_sb = stats.tile([8, 2], f32r, name="g_sb")
                tmp8 = stats.tile([8, 2], f32, name="tmp8")
                nc.vector.tensor_copy(out=g_sb[:, 0:1], in_=pgs[:, 0:1])
                nc.vector.tensor_mul(tmp8[:, 0:1], pgs[:, 0:1], pgs[:, 0:1])
                nc.vector.tensor_add(tmp8[:, 1:2], pgs[:, 1:2], pgs[:, 2:3])
                nc.vector.tensor_sub(tmp8[:, 1:2], tmp8[:, 1:2], tmp8[:, 0:1])
                nc.scalar.activation(
                    out=g_sb[:, 1:2], in_=tmp8[:, 1:2], func=Act.Sqrt, bias=eps_sb
                )
                with nc.allow_low_precision("fp22 matmul input rounding"):
                    nc.vector.reciprocal(out=g_sb[:, 1:2], in_=g_sb[:, 1:2])
                # broadcast group stats to channels: bc [128, {mean, rstd}]
                bct = mm1.tile([128, 512], f32, name="gn_ps", tag="mm")
                bc = bct[:, 0:2]
                nc.tensor.matmul(bc, lhsT=indb_sb, rhs=g_sb, start=True, stop=True)
                # sc cols: [posbias, scale];  xn = x*scale + posbias
                sc = stats.tile([128, 2], f32, name="sc")
                nc.vector.tensor_scalar_mul(sc[:, 1:2], bc[:, 1:2], gnw_sb[:, k:k + 1])
                nc.vector.tensor_mul(sc[:, 0:1], bc[:, 0:1], sc[:, 1:2])
                nc.vector.tensor_scalar(
                    sc[:, 0:1], sc[:, 0:1], gnb_sb[:, k:k + 1], None, op0=Alu.subtract
                )
                nc.vector.tensor_scalar_mul(sc[:, 0:1], sc[:, 0:1], -1.0)
                nc.scalar.activation(
                    out=xn_bf[k][:, b, :],
                    in_=xt,
                    func=Act.Identity,
                    bias=sc[:, 0:1],
                    scale=sc[:, 1:2],
                )

        q_sb = {}
        k_sb = {}
        vt_sb = {}
        ao_sb = {}

        def emit_qkv(b):
            # q, k: [128, head, 1024]; m-tile 0..3 -> q head, 4..7 -> k head
            q_sb[b] = qk_pool.tile([128, HEADS, S], bf16, name="q_sb")
            k_sb[b] = qk_pool.tile([128, HEADS, S], bf16, name="k_sb")
            for m in range(2 * HEADS):
                dst = q_sb[b] if m < HEADS else k_sb[b]
                for n in range(NH):
                    ps = mm1.tile([128, 512], f32, name="qk_ps", tag="mm")
                    for kk in range(CT):
                        nc.tensor.matmul(
                            ps,
                            lhsT=qkvT_sb[kk][:, m * 128:(m + 1) * 128],
                            rhs=xn_bf[kk][:, b, n * 512:(n + 1) * 512],
                            start=(kk == 0),
                            stop=(kk == CT - 1),
                        )
                    nc.vector.tensor_scalar_add(
                        dst[:, m % HEADS, n * 512:(n + 1) * 512],
                        ps,
                        qkvb_sb[:, m:m + 1],
                    )
            # v_T: [128(j), jt, 512(cv)]
            vt_sb[b] = vt_pool.tile([128, JT, C], bf16, name="vt_sb")
            for jt in range(JT):
                ps = mm1.tile([128, 512], f32, name="qk_ps", tag="mm")
                for kk in range(CT):
                    nc.tensor.matmul(
                        ps,
                        lhsT=xn_bf[kk][:, b, jt * 128:(jt + 1) * 128],
                        rhs=qkvT_sb[kk][:, 2 * C:3 * C],
                        start=(kk == 0),
                        stop=(kk == CT - 1),
                    )
                nc.vector.tensor_copy(out=vt_sb[b][:, jt, :], in_=ps)

        def emit_attn(b):
            ao_sb[b] = ao_pool.tile([128, HEADS, S], bf16, name="ao_sb")
            for h in range(HEADS):
                dn = dn_ps.tile([1, S], f32, name="dn")
                ot = o_ps.tile([128, S], f32, name="ot")
                ets = [None] * JT

                def dn_ot(jt):
                    for n in range(NH):
                        lo, hi = n * 512, (n + 1) * 512
                        nc.tensor.matmul(
                            dn[:, lo:hi],
                            lhsT=ones_bf[:, 0:1],
                            rhs=ets[jt][:, lo:hi],
                            start=(jt == 0),
                            stop=(jt == JT - 1),
                        )
                        nc.tensor.matmul(
                            ot[:, lo:hi],
                            lhsT=vt_sb[b][:, jt, h * 128:(h + 1) * 128],
                            rhs=ets[jt][:, lo:hi],
                            start=(jt == 0),
                            stop=(jt == JT - 1),
                        )

                # scores + exp run one j-tile ahead of denominator/output MMs
                for jt in range(JT):
                    et = e_pool.tile([128, S], bf16, name="et")
                    ets[jt] = et
                    for n in range(NH):
                        lo, hi = n * 512, (n + 1) * 512
                        sp = s_ps.tile([128, 512], f32, name="sp")
                        nc.tensor.matmul(
                            sp,
                            lhsT=k_sb[b][:, h, jt * 128:(jt + 1) * 128],
                            rhs=q_sb[b][:, h, lo:hi],
                            start=True,
                            stop=True,
                        )
                        nc.scalar.activation(
                            out=et[:, lo:hi], in_=sp, func=Act.Exp, scale=SCALE
                        )
                    if jt > 0:
                        dn_ot(jt - 1)
                dn_ot(JT - 1)

                # denominators -> SBUF, broadcast via GpSimd, reciprocal
                dns = stats.tile([1, S], f32, name="dns")
                nc.scalar.copy(out=dns, in_=dn)
                dcb = dcb_pool.tile([128, S], f32, name="dcb")
                nc.gpsimd.partition_broadcast(out_ap=dcb, in_ap=dns)
                rc = rc_pool.tile([128, S], f32, name="rc")
                nc.vector.reciprocal_approx_fast(out=rc, in_=dcb)
                # attnout = ot * rc + v_bias
                nc.vector.tensor_mul(ao_sb[b][:, h, :], ot, rc)
                nc.vector.tensor_scalar_add(
                    ao_sb[b][:, h, :], ao_sb[b][:, h, :], qkvb_sb[:, 8 + h:9 + h]
                )

        def emit_proj(b):
            for m in range(CT):
                for n in range(NH):
                    ps = mm1.tile([128, 512], f32, name="qk_ps", tag="mm")
                    for kk in range(CT):
                        nc.tensor.matmul(
                            ps,
                            lhsT=projT_sb[kk][:, m * 128:(m + 1) * 128],
                            rhs=ao_sb[b][:, kk, n * 512:(n + 1) * 512],
                            start=(kk == 0),
                            stop=(kk == CT - 1),
                        )
                    fo = fo_pool.tile([128, 512], f32, name="fo")
                    # fo = (ps + proj_b) + xn
                    nc.vector.affine_then_add(
                        out=fo,
                        in0=ps,
                        in1=xn_bf[m][:, b, n * 512:(n + 1) * 512],
                        scale=1.0,
                        bias=projb_sb[:, m:m + 1],
                    )
                    nc.sync.dma_start(
                        out=out_d[b, m * 128:(m + 1) * 128, n * 512:(n + 1) * 512],
                        in_=fo,
                    )

        emit_gn(0)
        emit_qkv(0)
        emit_gn(1)
        emit_attn(0)
        emit_qkv(1)
        emit_proj(0)
        emit_attn(1)
        emit_proj(1)


def _build_nc():
    import concourse.tile as tile
    from concourse import bacc, mybir

    f32 = mybir.dt.float32
    f32r = mybir.dt.float32r
    bf16 = mybir.dt.bfloat16
    nc = bacc.Bacc("TRN2", target_bir_lowering=False, debug=False)
    io = {
        "x": nc.dram_tensor("x", [BPC, C, S], f32, kind="ExternalInput").ap(),
        "qkvT": nc.dram_tensor("qkvT", [C, 3 * C], bf16, kind="ExternalInput").ap(),
        "projT": nc.dram_tensor("projT", [C, C], bf16, kind="ExternalInput").ap(),
        "qkvb": nc.dram_tensor("qkvb", [128, 12], f32, kind="ExternalInput").ap(),
        "gnw": nc.dram_tensor("gnw", [128, CT], f32, kind="ExternalInput").ap(),
        "gnb": nc.dram_tensor("gnb", [128, CT], f32, kind="ExternalInput").ap(),
        "projb": nc.dram_tensor("projb", [128, CT], f32, kind="ExternalInput").ap(),
        "indp": nc.dram_tensor("indp", [128, 8], f32r, kind="ExternalInput").ap(),
        "indb": nc.dram_tensor("indb", [8, 128], f32r, kind="ExternalInput").ap(),
        "out": nc.dram_tensor("out", [BPC, C, S], f32, kind="ExternalOutput").ap(),
    }
    with tile.TileContext(nc) as tc:
        _emit(tc, io)
    nc.compile()
    return nc


def get_nc():
    if "nc" not in _CACHE:
        _CACHE["nc"] = _build_nc()
    return _CACHE["nc"]


def make_const_inputs(norm_w, norm_b, qkv_w, qkv_b, proj_w, proj_b):
    """Host-side constant tensors shared by all cores."""
    import ml_dtypes

    f = np.float32
    bf = ml_dtypes.bfloat16
    qkvT = np.ascontiguousarray(qkv_w.T.astype(bf))            # [C, 3C]
    projT = np.ascontiguousarray(proj_w.T.astype(bf))          # [C, C]
    qkvb = np.ascontiguousarray(qkv_b.reshape(12, 128).T, dtype=f)
    gnw = np.ascontiguousarray(norm_w.reshape(CT, 128).T, dtype=f)
    gnb = np.ascontiguousarray(norm_b.reshape(CT, 128).T, dtype=f)
    projb = np.ascontiguousarray(proj_b.reshape(CT, 128).T, dtype=f)
    indp = np.zeros((128, 8), dtype=f)
    for p in range(128):
        indp[p, p // 16] = 1.0 / 16.0
    indb = np.zeros((8, 128), dtype=f)
    for p in range(128):
        indb[p // 16, p] = 1.0
    return {
        "qkvT": qkvT, "projT": projT, "qkvb": qkvb,
        "gnw": gnw, "gnb": gnb, "projb": projb,
        "indp": indp, "indb": indb,
    }


def kernel(x, norm_w, norm_b, qkv_w, qkv_b, proj_w, proj_b, _trace=False):
    from concourse.bass_utils import run_bass_kernel_spmd

    b, c, h, w = x.shape
    assert (b, c, h * w) == (B, C, S), f"unexpected input shape {x.shape}"
    consts = make_const_inputs(norm_w, norm_b, qkv_w, qkv_b, proj_w, proj_b)
    xf = np.ascontiguousarray(x.reshape(B, C, S), dtype=np.float32)
    in_maps = [
        {"x": np.ascontiguousarray(xf[i * BPC:(i + 1) * BPC]), **consts}
        for i in range(NCORES)
    ]
    nc = get_nc()
    res = run_bass_kernel_spmd(
        nc, in_maps, core_ids=list(range(NCORES)), trace=_trace
    )
    out = np.concatenate([r["out"] for r in res.results], axis=0)
    out = out.reshape(B, C, h, w).astype(np.float32)
    if _trace:
        _CACHE["last_results"] = res
    return out


# revision 11
# speedup vs baseline: 1.0943x; 1.0060x over previous
"""Trainium2 Bass kernel for GroupNorm + multi-head self-attention block.

Reference computation (per batch element):
    xn  = GroupNorm(x; 32 groups, eps=1e-5) * norm_w + norm_b
    qkv = qkv_w @ xn + qkv_b          (1x1 conv == channel matmul)
    q,k,v split; 4 heads of dh=128 over 1024 spatial positions
    attn = softmax(q^T k * C**-0.5); out = attn @ v
    out = proj_w @ out + proj_b + xn

Sharding: pure data-parallel over batch (16 batches / 8 cores = 2 per core),
no collectives.

Precision: GroupNorm statistics in fp32 (cross-partition pooling via small
fp32r indicator matmuls); the large matmuls (qkv, scores, softmax
denominator, attn*v, proj) in bf16 with fp32 PSUM accumulation; softmax and
bias/residual arithmetic in fp32.

Schedule highlights:
  - GroupNorm runs per batch so batch 0's qkv matmuls start while batch 1's
    stats are still on the Vector engine.
  - Attention is software-pipelined: denominator/output matmuls trail the
    score matmuls by one j-tile so ScalarE exp latency stays off the PE
    critical path.
  - softmax reciprocal broadcast runs on GpSimd (partition_broadcast).
"""

from contextlib import ExitStack

import numpy as np

B = 16          # full batch
C = 512         # channels
S = 1024        # spatial (32*32)
HEADS = 4
DH = C // HEADS         # 128, head dim == partition tile
GROUPS = 32
EPS = 1e-5
NCORES = 8
BPC = B // NCORES       # 2 batches per core
CT = C // 128           # 4 channel tiles
SCALE = float(C) ** -0.5
JT = S // 128           # 8 j-tiles (key positions)
NH = S // 512           # 2 free-dim halves

_CACHE = {}


def _emit(tc, io):
    from concourse import mybir

    nc = tc.nc
    f32 = mybir.dt.float32
    f32r = mybir.dt.float32r
    bf16 = mybir.dt.bfloat16
    Act = mybir.ActivationFunctionType
    Alu = mybir.AluOpType

    x_d = io["x"]
    out_d = io["out"]

    with ExitStack() as ctx:
        consts = ctx.enter_context(tc.tile_pool(name="consts", bufs=1))
        x_pool = ctx.enter_context(tc.tile_pool(name="x_pool", bufs=3))
        xnbf_pool = ctx.enter_context(tc.tile_pool(name="xnbf_pool", bufs=1))
        stats = ctx.enter_context(tc.tile_pool(name="stats", bufs=4))
        qk_pool = ctx.enter_context(tc.tile_pool(name="qk_pool", bufs=2))
        vt_pool = ctx.enter_context(tc.tile_pool(name="vt_pool", bufs=2))
        ao_pool = ctx.enter_context(tc.tile_pool(name="ao_pool", bufs=2))
        e_pool = ctx.enter_context(tc.tile_pool(name="e_pool", bufs=6))
        rc_pool = ctx.enter_context(tc.tile_pool(name="rc_pool", bufs=2))
        dcb_pool = ctx.enter_context(tc.tile_pool(name="dcb_pool", bufs=2))
        fo_pool = ctx.enter_context(tc.tile_pool(name="fo_pool", bufs=4))
        # PSUM pools: mm1(2) + s(2) + o(2) + dn(2) = 8 banks
        mm1 = ctx.enter_context(tc.tile_pool(name="mm1", bufs=2, space="PSUM"))
        s_ps = ctx.enter_context(tc.tile_pool(name="s_ps", bufs=2, space="PSUM"))
        o_ps = ctx.enter_context(tc.tile_pool(name="o_ps", bufs=1, space="PSUM"))
        dn_ps = ctx.enter_context(tc.tile_pool(name="dn_ps", bufs=1, space="PSUM"))

        # ---- constants ----
        qkvT_sb = []
        for k in range(CT):
            t = consts.tile([128, 3 * C], bf16, name=f"qkvT{k}")
            nc.sync.dma_start(out=t, in_=io["qkvT"][k * 128:(k + 1) * 128, :])
            qkvT_sb.append(t)
        projT_sb = []
        for k in range(CT):
            t = consts.tile([128, C], bf16, name=f"projT{k}")
            nc.sync.dma_start(out=t, in_=io["projT"][k * 128:(k + 1) * 128, :])
            projT_sb.append(t)
        qkvb_sb = consts.tile([128, 12], f32, name="qkvb_sb")
        nc.sync.dma_start(out=qkvb_sb, in_=io["qkvb"])
        gnw_sb = consts.tile([128, CT], f32, name="gnw_sb")
        nc.sync.dma_start(out=gnw_sb, in_=io["gnw"])
        gnb_sb = consts.tile([128, CT], f32, name="gnb_sb")
        nc.sync.dma_start(out=gnb_sb, in_=io["gnb"])
        projb_sb = consts.tile([128, CT], f32, name="projb_sb")
        nc.sync.dma_start(out=projb_sb, in_=io["projb"])
        indp_sb = consts.tile([128, 8], f32r, name="indp_sb")
        nc.sync.dma_start(out=indp_sb, in_=io["indp"])
        indb_sb = consts.tile([8, 128], f32r, name="indb_sb")
        nc.sync.dma_start(out=indb_sb, in_=io["indb"])
        ones_bf = consts.tile([128, 128], bf16, name="ones_bf")
        nc.vector.memset(ones_bf, 1.0)
        eps_sb = consts.tile([8, 1], f32, name="eps_sb")
        nc.vector.memset(eps_sb, EPS)

        # normalized x in bf16, per batch: [128, 1024] per (ctile, b)
        xn_bf = [
            xnbf_pool.tile([128, BPC, S], bf16, name=f"xnbf{k}") for k in range(CT)
        ]

        def emit_gn(b):
            """GroupNorm for batch b -> xn_bf[k][:, b, :]."""
            for k in range(CT):
                xt = x_pool.tile([128, S], f32, name="xt")
                nc.sync.dma_start(out=xt, in_=x_d[b, k * 128:(k + 1) * 128, :])
                # per-channel mean / var / mean^2
                sb_stf = stats.tile([128, 4], f32, name="sb_stf")
                sb_st = stats.tile([128, 4], f32r, name="sb_st")
                bn6 = stats.tile([128, 2, 6], f32, name="bn6")
                for u in range(2):
                    nc.vector.bn_stats(
                        out=bn6[:, u, :], in_=xt[:, u * 512:(u + 1) * 512]
                    )
                nc.vector.bn_aggr(out=sb_stf[:, 0:2], in_=bn6)
                nc.vector.tensor_mul(sb_stf[:, 2:3], sb_stf[:, 0:1], sb_stf[:, 0:1])
                nc.vector.tensor_copy(out=sb_stf[:, 3:4], in_=sb_stf[:, 0:1])
                nc.vector.tensor_copy(out=sb_st, in_=sb_stf)
                # pool over 16-channel groups (x 1/16): pg[g, {mean, var, mean2, pad}]
                pgt = mm1.tile([128, 512], f32, name="gn_ps", tag="mm")
                pg = pgt[0:8, 0:4]
                nc.tensor.matmul(pg, lhsT=indp_sb, rhs=sb_st, start=True, stop=True)
                pgs = stats.tile([8, 4], f32, name="pgs")
                nc.vector.tensor_copy(out=pgs, in_=pg)
                # g_sb cols: [mean_g, rstd_g]
                g_sb = stats.tile([8, 2], f32r, name="g_sb")
                tmp8 = stats.tile([8, 2], f32, name="tmp8")
                nc.vector.tensor_copy(out=g_sb[:, 0:1], in_=pgs[:, 0:1])
                nc.vector.tensor_mul(tmp8[:, 0:1], pgs[:, 0:1], pgs[:, 0:1])
                nc.vector.tensor_add(tmp8[:, 1:2], pgs[:, 1:2], pgs[:, 2:3])
                nc.vector.tensor_sub(tmp8[:, 1:2], tmp8[:, 1:2], tmp8[:, 0:1])
                nc.scalar.activation(
                    out=g_sb[:, 1:2], in_=tmp8[:, 1:2], func=Act.Sqrt, bias=eps_sb
                )
                with nc.allow_low_precision("fp22 matmul input rounding"):
                    nc.vector.reciprocal(out=g_sb[:, 1:2], in_=g_sb[:, 1:2])
                # broadcast group stats to channels: bc [128, {mean, rstd}]
                bct = mm1.tile([128, 512], f32, name="gn_ps", tag="mm")
                bc = bct[:, 0:2]
                nc.tensor.matmul(bc, lhsT=indb_sb, rhs=g_sb, start=True, stop=True)
                # sc cols: [posbias, scale];  xn = x*scale + posbias
                sc = stats.tile([128, 2], f32, name="sc")
                nc.vector.tensor_scalar_mul(sc[:, 1:2], bc[:, 1:2], gnw_sb[:, k:k + 1])
                nc.vector.tensor_mul(sc[:, 0:1], bc[:, 0:1], sc[:, 1:2])
                nc.vector.tensor_scalar(
                    sc[:, 0:1], sc[:, 0:1], gnb_sb[:, k:k + 1], None, op0=Alu.subtract
                )
                nc.vector.tensor_scalar_mul(sc[:, 0:1], sc[:, 0:1], -1.0)
                nc.scalar.activation(
                    out=xn_bf[k][:, b, :],
                    in_=xt,
                    func=Act.Identity,
                    bias=sc[:, 0:1],
                    scale=sc[:, 1:2],
                )

        q_sb = {}
        k_sb = {}
        vt_sb = {}
        ao_sb = {}

        def emit_qkv(b):
            # q, k: [128, head, 1024]; m-tile 0..3 -> q head, 4..7 -> k head
            q_sb[b] = qk_pool.tile([128, HEADS, S], bf16, name="q_sb")
            k_sb[b] = qk_pool.tile([128, HEADS, S], bf16, name="k_sb")
            for m in range(2 * HEADS):
                dst = q_sb[b] if m < HEADS else k_sb[b]
                for n in range(NH):
                    ps = mm1.tile([128, 512], f32, name="qk_ps", tag="mm")
                    for kk in range(CT):
                        nc.tensor.matmul(
                            ps,
                            lhsT=qkvT_sb[kk][:, m * 128:(m + 1) * 128],
                            rhs=xn_bf[kk][:, b, n * 512:(n + 1) * 512],
                            start=(kk == 0),
                            stop=(kk == CT - 1),
                        )
                    nc.vector.tensor_scalar_add(
                        dst[:, m % HEADS, n * 512:(n + 1) * 512],
                        ps,
                        qkvb_sb[:, m:m + 1],
                    )
            # v_T: [128(j), jt, 512(cv)]
            vt_sb[b] = vt_pool.tile([128, JT, C], bf16, name="vt_sb")
            for jt in range(JT):
                ps = mm1.tile([128, 512], f32, name="qk_ps", tag="mm")
                for kk in range(CT):
                    nc.tensor.matmul(
                        ps,
                        lhsT=xn_bf[kk][:, b, jt * 128:(jt + 1) * 128],
                        rhs=qkvT_sb[kk][:, 2 * C:3 * C],
                        start=(kk == 0),
                        stop=(kk == CT - 1),
                    )
                nc.vector.tensor_copy(out=vt_sb[b][:, jt, :], in_=ps)

        def emit_attn(b):
            ao_sb[b] = ao_pool.tile([128, HEADS, S], bf16, name="ao_sb")
            for h in range(HEADS):
                dn = dn_ps.tile([1, S], f32, name="dn")
                ot = o_ps.tile([128, S], f32, name="ot")
                ets = [None] * JT

                def dn_ot(jt):
                    for n in range(NH):
                        lo, hi = n * 512, (n + 1) * 512
                        nc.tensor.matmul(
                            dn[:, lo:hi],
                            lhsT=ones_bf[:, 0:1],
                            rhs=ets[jt][:, lo:hi],
                            start=(jt == 0),
                            stop=(jt == JT - 1),
                        )
                        nc.tensor.matmul(
                            ot[:, lo:hi],
                            lhsT=vt_sb[b][:, jt, h * 128:(h + 1) * 128],
                            rhs=ets[jt][:, lo:hi],
                            start=(jt == 0),
                            stop=(jt == JT - 1),
                        )

                # scores + exp run one j-tile ahead of denominator/output MMs
                for jt in range(JT):
                    et = e_pool.tile([128, S], bf16, name="et")
                    ets[jt] = et
                    for n in range(NH):
                        lo, hi = n * 512, (n + 1) * 512
                        sp = s_ps.tile([128, 512], f32, name="sp")
                        nc.tensor.matmul(
                            sp,
                            lhsT=k_sb[b][:, h, jt * 128:(jt + 1) * 128],
                            rhs=q_sb[b][:, h, lo:hi],
                            start=True,
                            stop=True,
                        )
                        nc.scalar.activation(
                            out=et[:, lo:hi], in_=sp, func=Act.Exp, scale=SCALE
                        )
                    if jt > 1:
                        dn_ot(jt - 2)
                dn_ot(JT - 2)
                dn_ot(JT - 1)

                # free the PSUM accumulator early; epilogue reads the copy
                ot_sb = rc_pool.tile([128, S], f32, name="ot_sb")
                nc.vector.tensor_copy(out=ot_sb, in_=ot)
                # denominators -> SBUF, broadcast via GpSimd, reciprocal
                dns = stats.tile([1, S], f32, name="dns")
                nc.scalar.copy(out=dns, in_=dn)
                dcb = dcb_pool.tile([128, S], f32, name="dcb")
                nc.gpsimd.partition_broadcast(out_ap=dcb, in_ap=dns)
                rc = rc_pool.tile([128, S], f32, name="rc")
                nc.vector.reciprocal_approx_fast(out=rc, in_=dcb)
                # attnout = ot * rc + v_bias
                nc.vector.tensor_mul(ao_sb[b][:, h, :], ot_sb, rc)
                nc.vector.tensor_scalar_add(
                    ao_sb[b][:, h, :], ao_sb[b][:, h, :], qkvb_sb[:, 8 + h:9 + h]
                )

        def emit_proj(b):
            for m in range(CT):
                for n in range(NH):
                    ps = mm1.tile([128, 512], f32, name="qk_ps", tag="mm")
                    for kk in range(CT):
                        nc.tensor.matmul(
                            ps,
                            lhsT=projT_sb[kk][:, m * 128:(m + 1) * 128],
                            rhs=ao_sb[b][:, kk, n * 512:(n + 1) * 512],
                            start=(kk == 0),
                            stop=(kk == CT - 1),
                        )
                    fo = fo_pool.tile([128, 512], f32, name="fo")
                    # fo = (ps + proj_b) + xn
                    nc.vector.affine_then_add(
                        out=fo,
                        in0=ps,
                        in1=xn_bf[m][:, b, n * 512:(n + 1) * 512],
                        scale=1.0,
                        bias=projb_sb[:, m:m + 1],
                    )
                    nc.sync.dma_start(
                        out=out_d[b, m * 128:(m + 1) * 128, n * 512:(n + 1) * 512],
                        in_=fo,
                    )

        emit_gn(0)
        emit_qkv(0)
        emit_gn(1)
        emit_attn(0)
        emit_qkv(1)
        emit_proj(0)
        emit_attn(1)
        emit_proj(1)


def _build_nc():
    import concourse.tile as tile
    from concourse import bacc, mybir

    f32 = mybir.dt.float32
    f32r = mybir.dt.float32r
    bf16 = mybir.dt.bfloat16
    nc = bacc.Bacc("TRN2", target_bir_lowering=False, debug=False)
    io = {
        "x": nc.dram_tensor("x", [BPC, C, S], f32, kind="ExternalInput").ap(),
        "qkvT": nc.dram_tensor("qkvT", [C, 3 * C], bf16, kind="ExternalInput").ap(),
        "projT": nc.dram_tensor("projT", [C, C], bf16, kind="ExternalInput").ap(),
        "qkvb": nc.dram_tensor("qkvb", [128, 12], f32, kind="ExternalInput").ap(),
        "gnw": nc.dram_tensor("gnw", [128, CT], f32, kind="ExternalInput").ap(),
        "gnb": nc.dram_tensor("gnb", [128, CT], f32, kind="ExternalInput").ap(),
        "projb": nc.dram_tensor("projb", [128, CT], f32, kind="ExternalInput").ap(),
        "indp": nc.dram_tensor("indp", [128, 8], f32r, kind="ExternalInput").ap(),
        "indb": nc.dram_tensor("indb", [8, 128], f32r, kind="ExternalInput").ap(),
        "out": nc.dram_tensor("out", [BPC, C, S], f32, kind="ExternalOutput").ap(),
    }
    with tile.TileContext(nc) as tc:
        _emit(tc, io)
    nc.compile()
    return nc


def get_nc():
    if "nc" not in _CACHE:
        _CACHE["nc"] = _build_nc()
    return _CACHE["nc"]


def make_const_inputs(norm_w, norm_b, qkv_w, qkv_b, proj_w, proj_b):
    """Host-side constant tensors shared by all cores."""
    import ml_dtypes

    f = np.float32
    bf = ml_dtypes.bfloat16
    qkvT = np.ascontiguousarray(qkv_w.T.astype(bf))            # [C, 3C]
    projT = np.ascontiguousarray(proj_w.T.astype(bf))          # [C, C]
    qkvb = np.ascontiguousarray(qkv_b.reshape(12, 128).T, dtype=f)
    gnw = np.ascontiguousarray(norm_w.reshape(CT, 128).T, dtype=f)
    gnb = np.ascontiguousarray(norm_b.reshape(CT, 128).T, dtype=f)
    projb = np.ascontiguousarray(proj_b.reshape(CT, 128).T, dtype=f)
    indp = np.zeros((128, 8), dtype=f)
    for p in range(128):
        indp[p, p // 16] = 1.0 / 16.0
    indb = np.zeros((8, 128), dtype=f)
    for p in range(128):
        indb[p // 16, p] = 1.0
    return {
        "qkvT": qkvT, "projT": projT, "qkvb": qkvb,
        "gnw": gnw, "gnb": gnb, "projb": projb,
        "indp": indp, "indb": indb,
    }


def kernel(x, norm_w, norm_b, qkv_w, qkv_b, proj_w, proj_b, _trace=False):
    from concourse.bass_utils import run_bass_kernel_spmd

    b, c, h, w = x.shape
    assert (b, c, h * w) == (B, C, S), f"unexpected input shape {x.shape}"
    consts = make_const_inputs(norm_w, norm_b, qkv_w, qkv_b, proj_w, proj_b)
    xf = np.ascontiguousarray(x.reshape(B, C, S), dtype=np.float32)
    in_maps = [
        {"x": np.ascontiguousarray(xf[i * BPC:(i + 1) * BPC]), **consts}
        for i in range(NCORES)
    ]
    nc = get_nc()
    res = run_bass_kernel_spmd(
        nc, in_maps, core_ids=list(range(NCORES)), trace=_trace
    )
    out = np.concatenate([r["out"] for r in res.results], axis=0)
    out = out.reshape(B, C, h, w).astype(np.float32)
    if _trace:
        _CACHE["last_results"] = res
    return out


# revision 12
# speedup vs baseline: 1.2750x; 1.1652x over previous
"""Trainium2 Bass kernel for GroupNorm + multi-head self-attention block.

Reference computation (per batch element):
    xn  = GroupNorm(x; 32 groups, eps=1e-5) * norm_w + norm_b
    qkv = qkv_w @ xn + qkv_b          (1x1 conv == channel matmul)
    q,k,v split; 4 heads of dh=128 over 1024 spatial positions
    attn = softmax(q^T k * C**-0.5); out = attn @ v
    out = proj_w @ out + proj_b + xn

Sharding: pure data-parallel over batch (16 batches / 8 cores = 2 per core),
no collectives.

Precision: GroupNorm statistics in fp32 (cross-partition pooling via small
fp32r indicator matmuls); the large matmuls (qkv, scores, softmax
denominator, attn*v, proj) in bf16 with fp32 PSUM accumulation; softmax and
bias/residual arithmetic in fp32.

Schedule highlights:
  - GroupNorm runs per batch so batch 0's qkv matmuls start while batch 1's
    stats are still on the Vector engine.
  - Attention is software-pipelined: denominator/output matmuls trail the
    score matmuls by one j-tile so ScalarE exp latency stays off the PE
    critical path.
  - softmax reciprocal broadcast runs on GpSimd (partition_broadcast).
"""

from contextlib import ExitStack

import numpy as np

B = 16          # full batch
C = 512         # channels
S = 1024        # spatial (32*32)
HEADS = 4
DH = C // HEADS         # 128, head dim == partition tile
GROUPS = 32
EPS = 1e-5
NCORES = 8
BPC = B // NCORES       # 2 batches per core
CT = C // 128           # 4 channel tiles
SCALE = float(C) ** -0.5
JT = S // 128           # 8 j-tiles (key positions)
NH = S // 512           # 2 free-dim halves

_CACHE = {}


def _emit(tc, io):
    from concourse import mybir

    nc = tc.nc
    f32 = mybir.dt.float32
    f32r = mybir.dt.float32r
    bf16 = mybir.dt.bfloat16
    Act = mybir.ActivationFunctionType
    Alu = mybir.AluOpType

    x_d = io["x"]
    out_d = io["out"]

    with ExitStack() as ctx:
        consts = ctx.enter_context(tc.tile_pool(name="consts", bufs=1))
        x_pool = ctx.enter_context(tc.tile_pool(name="x_pool", bufs=3))
        xnbf_pool = ctx.enter_context(tc.tile_pool(name="xnbf_pool", bufs=1))
        stats = ctx.enter_context(tc.tile_pool(name="stats", bufs=4))
        qk_pool = ctx.enter_context(tc.tile_pool(name="qk_pool", bufs=2))
        vt_pool = ctx.enter_context(tc.tile_pool(name="vt_pool", bufs=2))
        ao_pool = ctx.enter_context(tc.tile_pool(name="ao_pool", bufs=2))
        e_pool = ctx.enter_context(tc.tile_pool(name="e_pool", bufs=6))
        rc_pool = ctx.enter_context(tc.tile_pool(name="rc_pool", bufs=2))
        fo_pool = ctx.enter_context(tc.tile_pool(name="fo_pool", bufs=4))
        # PSUM pools: mm1(2) + s(2) + o(2) + dn(2) = 8 banks
        mm1 = ctx.enter_context(tc.tile_pool(name="mm1", bufs=2, space="PSUM"))
        s_ps = ctx.enter_context(tc.tile_pool(name="s_ps", bufs=2, space="PSUM"))
        o_ps = ctx.enter_context(tc.tile_pool(name="o_ps", bufs=1, space="PSUM"))
        dn_ps = ctx.enter_context(tc.tile_pool(name="dn_ps", bufs=1, space="PSUM"))

        # ---- constants ----
        qkvT_sb = []
        for k in range(CT):
            t = consts.tile([128, 3 * C], bf16, name=f"qkvT{k}")
            nc.sync.dma_start(out=t, in_=io["qkvT"][k * 128:(k + 1) * 128, :])
            qkvT_sb.append(t)
        projT_sb = []
        for k in range(CT):
            t = consts.tile([128, C], bf16, name=f"projT{k}")
            nc.sync.dma_start(out=t, in_=io["projT"][k * 128:(k + 1) * 128, :])
            projT_sb.append(t)
        qkvb_sb = consts.tile([128, 12], f32, name="qkvb_sb")
        nc.sync.dma_start(out=qkvb_sb, in_=io["qkvb"])
        gnw_sb = consts.tile([128, CT], f32, name="gnw_sb")
        nc.sync.dma_start(out=gnw_sb, in_=io["gnw"])
        gnb_sb = consts.tile([128, CT], f32, name="gnb_sb")
        nc.sync.dma_start(out=gnb_sb, in_=io["gnb"])
        projb_sb = consts.tile([128, CT], f32, name="projb_sb")
        nc.sync.dma_start(out=projb_sb, in_=io["projb"])
        indp_sb = consts.tile([128, 8], f32r, name="indp_sb")
        nc.sync.dma_start(out=indp_sb, in_=io["indp"])
        indb_sb = consts.tile([8, 128], f32r, name="indb_sb")
        nc.sync.dma_start(out=indb_sb, in_=io["indb"])
        ones_bf = consts.tile([128, 128], bf16, name="ones_bf")
        nc.vector.memset(ones_bf, 1.0)
        eps_sb = consts.tile([8, 1], f32, name="eps_sb")
        nc.vector.memset(eps_sb, EPS)

        # normalized x in bf16, per batch: [128, 1024] per (ctile, b)
        xn_bf = [
            xnbf_pool.tile([128, BPC, S], bf16, name=f"xnbf{k}") for k in range(CT)
        ]

        def emit_gn(b):
            """GroupNorm for batch b -> xn_bf[k][:, b, :]."""
            for k in range(CT):
                xt = x_pool.tile([128, S], f32, name="xt")
                nc.sync.dma_start(out=xt, in_=x_d[b, k * 128:(k + 1) * 128, :])
                # per-channel mean / var / mean^2
                sb_stf = stats.tile([128, 4], f32, name="sb_stf")
                sb_st = stats.tile([128, 4], f32r, name="sb_st")
                bn6 = stats.tile([128, 2, 6], f32, name="bn6")
                for u in range(2):
                    nc.vector.bn_stats(
                        out=bn6[:, u, :], in_=xt[:, u * 512:(u + 1) * 512]
                    )
                nc.vector.bn_aggr(out=sb_stf[:, 0:2], in_=bn6)
                nc.vector.tensor_mul(sb_stf[:, 2:3], sb_stf[:, 0:1], sb_stf[:, 0:1])
                nc.vector.tensor_copy(out=sb_stf[:, 3:4], in_=sb_stf[:, 0:1])
                nc.vector.tensor_copy(out=sb_st, in_=sb_stf)
                # pool over 16-channel groups (x 1/16): pg[g, {mean, var, mean2, pad}]
                pgt = mm1.tile([128, 512], f32, name="gn_ps", tag="mm")
                pg = pgt[0:8, 0:4]
                nc.tensor.matmul(pg, lhsT=indp_sb, rhs=sb_st, start=True, stop=True)
                pgs = stats.tile([8, 4], f32, name="pgs")
                nc.vector.tensor_copy(out=pgs, in_=pg)
                # g_sb cols: [mean_g, rstd_g]
                g_sb = stats.tile([8, 2], f32r, name="g_sb")
                tmp8 = stats.tile([8, 2], f32, name="tmp8")
                nc.vector.tensor_copy(out=g_sb[:, 0:1], in_=pgs[:, 0:1])
                nc.vector.tensor_mul(tmp8[:, 0:1], pgs[:, 0:1], pgs[:, 0:1])
                nc.vector.tensor_add(tmp8[:, 1:2], pgs[:, 1:2], pgs[:, 2:3])
                nc.vector.tensor_sub(tmp8[:, 1:2], tmp8[:, 1:2], tmp8[:, 0:1])
                nc.scalar.activation(
                    out=g_sb[:, 1:2], in_=tmp8[:, 1:2], func=Act.Sqrt, bias=eps_sb
                )
                with nc.allow_low_precision("fp22 matmul input rounding"):
                    nc.vector.reciprocal(out=g_sb[:, 1:2], in_=g_sb[:, 1:2])
                # broadcast group stats to channels: bc [128, {mean, rstd}]
                bct = mm1.tile([128, 512], f32, name="gn_ps", tag="mm")
                bc = bct[:, 0:2]
                nc.tensor.matmul(bc, lhsT=indb_sb, rhs=g_sb, start=True, stop=True)
                # sc cols: [posbias, scale];  xn = x*scale + posbias
                sc = stats.tile([128, 2], f32, name="sc")
                nc.vector.tensor_scalar_mul(sc[:, 1:2], bc[:, 1:2], gnw_sb[:, k:k + 1])
                nc.vector.tensor_mul(sc[:, 0:1], bc[:, 0:1], sc[:, 1:2])
                nc.vector.tensor_scalar(
                    sc[:, 0:1], sc[:, 0:1], gnb_sb[:, k:k + 1], None, op0=Alu.subtract
                )
                nc.vector.tensor_scalar_mul(sc[:, 0:1], sc[:, 0:1], -1.0)
                nc.scalar.activation(
                    out=xn_bf[k][:, b, :],
                    in_=xt,
                    func=Act.Identity,
                    bias=sc[:, 0:1],
                    scale=sc[:, 1:2],
                )

        q_sb = {}
        k_sb = {}
        vt_sb = {}
        ao_sb = {}

        def emit_qkv(b):
            # q, k: [128, head, 1024]; m-tile 0..3 -> q head, 4..7 -> k head
            q_sb[b] = qk_pool.tile([128, HEADS, S], bf16, name="q_sb")
            k_sb[b] = qk_pool.tile([128, HEADS, S], bf16, name="k_sb")
            for m in range(2 * HEADS):
                dst = q_sb[b] if m < HEADS else k_sb[b]
                for n in range(NH):
                    ps = mm1.tile([128, 512], f32, name="qk_ps", tag="mm")
                    for kk in range(CT):
                        nc.tensor.matmul(
                            ps,
                            lhsT=qkvT_sb[kk][:, m * 128:(m + 1) * 128],
                            rhs=xn_bf[kk][:, b, n * 512:(n + 1) * 512],
                            start=(kk == 0),
                            stop=(kk == CT - 1),
                        )
                    nc.vector.tensor_scalar_add(
                        dst[:, m % HEADS, n * 512:(n + 1) * 512],
                        ps,
                        qkvb_sb[:, m:m + 1],
                    )
            # v_T: [128(j), jt, 512(cv)]
            vt_sb[b] = vt_pool.tile([128, JT, C], bf16, name="vt_sb")
            for jt in range(JT):
                ps = mm1.tile([128, 512], f32, name="qk_ps", tag="mm")
                for kk in range(CT):
                    nc.tensor.matmul(
                        ps,
                        lhsT=xn_bf[kk][:, b, jt * 128:(jt + 1) * 128],
                        rhs=qkvT_sb[kk][:, 2 * C:3 * C],
                        start=(kk == 0),
                        stop=(kk == CT - 1),
                    )
                nc.vector.tensor_copy(out=vt_sb[b][:, jt, :], in_=ps)

        def emit_attn(b):
            ao_sb[b] = ao_pool.tile([128, HEADS, S], bf16, name="ao_sb")
            for h in range(HEADS):
                dn = dn_ps.tile([128, S], f32, name="dn")
                ot = o_ps.tile([128, S], f32, name="ot")
                ets = [None] * JT

                def dn_ot(jt):
                    for n in range(NH):
                        lo, hi = n * 512, (n + 1) * 512
                        nc.tensor.matmul(
                            dn[:, lo:hi],
                            lhsT=ones_bf,
                            rhs=ets[jt][:, lo:hi],
                            start=(jt == 0),
                            stop=(jt == JT - 1),
                        )
                        nc.tensor.matmul(
                            ot[:, lo:hi],
                            lhsT=vt_sb[b][:, jt, h * 128:(h + 1) * 128],
                            rhs=ets[jt][:, lo:hi],
                            start=(jt == 0),
                            stop=(jt == JT - 1),
                        )

                # scores + exp run one j-tile ahead of denominator/output MMs
                for jt in range(JT):
                    et = e_pool.tile([128, S], bf16, name="et")
                    ets[jt] = et
                    for n in range(NH):
                        lo, hi = n * 512, (n + 1) * 512
                        sp = s_ps.tile([128, 512], f32, name="sp")
                        nc.tensor.matmul(
                            sp,
                            lhsT=k_sb[b][:, h, jt * 128:(jt + 1) * 128],
                            rhs=q_sb[b][:, h, lo:hi],
                            start=True,
                            stop=True,
                        )
                        nc.scalar.activation(
                            out=et[:, lo:hi], in_=sp, func=Act.Exp, scale=SCALE
                        )
                    if jt > 1:
                        dn_ot(jt - 2)
                dn_ot(JT - 2)
                dn_ot(JT - 1)

                # dn already holds the denominator on every partition
                rc = rc_pool.tile([128, S], f32, name="rc")
                nc.vector.reciprocal_approx_fast(out=rc, in_=dn)
                # attnout = ot * rc + v_bias
                nc.vector.tensor_mul(ao_sb[b][:, h, :], ot, rc)
                nc.vector.tensor_scalar_add(
                    ao_sb[b][:, h, :], ao_sb[b][:, h, :], qkvb_sb[:, 8 + h:9 + h]
                )

        def emit_proj(b):
            for m in range(CT):
                for n in range(NH):
                    ps = mm1.tile([128, 512], f32, name="qk_ps", tag="mm")
                    for kk in range(CT):
                        nc.tensor.matmul(
                            ps,
                            lhsT=projT_sb[kk][:, m * 128:(m + 1) * 128],
                            rhs=ao_sb[b][:, kk, n * 512:(n + 1) * 512],
                            start=(kk == 0),
                            stop=(kk == CT - 1),
                        )
                    fo = fo_pool.tile([128, 512], f32, name="fo")
                    # fo = (ps + proj_b) + xn
                    nc.vector.affine_then_add(
                        out=fo,
                        in0=ps,
                        in1=xn_bf[m][:, b, n * 512:(n + 1) * 512],
                        scale=1.0,
                        bias=projb_sb[:, m:m + 1],
                    )
                    nc.sync.dma_start(
                        out=out_d[b, m * 128:(m + 1) * 128, n * 512:(n + 1) * 512],
                        in_=fo,
                    )

        emit_gn(0)
        emit_gn(1)
        emit_qkv(0)
        emit_attn(0)
        emit_qkv(1)
        emit_proj(0)
        emit_attn(1)
        emit_proj(1)


def _build_nc():
    import concourse.tile as tile
    from concourse import bacc, mybir

    f32 = mybir.dt.float32
    f32r = mybir.dt.float32r
    bf16 = mybir.dt.bfloat16
    nc = bacc.Bacc("TRN2", target_bir_lowering=False, debug=False)
    io = {
        "x": nc.dram_tensor("x", [BPC, C, S], f32, kind="ExternalInput").ap(),
        "qkvT": nc.dram_tensor("qkvT", [C, 3 * C], bf16, kind="ExternalInput").ap(),
        "projT": nc.dram_tensor("projT", [C, C], bf16, kind="ExternalInput").ap(),
        "qkvb": nc.dram_tensor("qkvb", [128, 12], f32, kind="ExternalInput").ap(),
        "gnw": nc.dram_tensor("gnw", [128, CT], f32, kind="ExternalInput").ap(),
        "gnb": nc.dram_tensor("gnb", [128, CT], f32, kind="ExternalInput").ap(),
        "projb": nc.dram_tensor("projb", [128, CT], f32, kind="ExternalInput").ap(),
        "indp": nc.dram_tensor("indp", [128, 8], f32r, kind="ExternalInput").ap(),
        "indb": nc.dram_tensor("indb", [8, 128], f32r, kind="ExternalInput").ap(),
        "out": nc.dram_tensor("out", [BPC, C, S], f32, kind="ExternalOutput").ap(),
    }
    with tile.TileContext(nc) as tc:
        _emit(tc, io)
    nc.compile()
    return nc


def get_nc():
    if "nc" not in _CACHE:
        _CACHE["nc"] = _build_nc()
    return _CACHE["nc"]


def make_const_inputs(norm_w, norm_b, qkv_w, qkv_b, proj_w, proj_b):
    """Host-side constant tensors shared by all cores."""
    import ml_dtypes

    f = np.float32
    bf = ml_dtypes.bfloat16
    qkvT = np.ascontiguousarray(qkv_w.T.astype(bf))            # [C, 3C]
    projT = np.ascontiguousarray(proj_w.T.astype(bf))          # [C, C]
    qkvb = np.ascontiguousarray(qkv_b.reshape(12, 128).T, dtype=f)
    gnw = np.ascontiguousarray(norm_w.reshape(CT, 128).T, dtype=f)
    gnb = np.ascontiguousarray(norm_b.reshape(CT, 128).T, dtype=f)
    projb = np.ascontiguousarray(proj_b.reshape(CT, 128).T, dtype=f)
    indp = np.zeros((128, 8), dtype=f)
    for p in range(128):
        indp[p, p // 16] = 1.0 / 16.0
    indb = np.zeros((8, 128), dtype=f)
    for p in range(128):
        indb[p // 16, p] = 1.0
    return {
        "qkvT": qkvT, "projT": projT, "qkvb": qkvb,
        "gnw": gnw, "gnb": gnb, "projb": projb,
        "indp": indp, "indb": indb,
    }


def kernel(x, norm_w, norm_b, qkv_w, qkv_b, proj_w, proj_b, _trace=False):
    from concourse.bass_utils import run_bass_kernel_spmd

    b, c, h, w = x.shape
    assert (b, c, h * w) == (B, C, S), f"unexpected input shape {x.shape}"
    consts = make_const_inputs(norm_w, norm_b, qkv_w, qkv_b, proj_w, proj_b)
    xf = np.ascontiguousarray(x.reshape(B, C, S), dtype=np.float32)
    in_maps = [
        {"x": np.ascontiguousarray(xf[i * BPC:(i + 1) * BPC]), **consts}
        for i in range(NCORES)
    ]
    nc = get_nc()
    res = run_bass_kernel_spmd(
        nc, in_maps, core_ids=list(range(NCORES)), trace=_trace
    )
    out = np.concatenate([r["out"] for r in res.results], axis=0)
    out = out.reshape(B, C, h, w).astype(np.float32)
    if _trace:
        _CACHE["last_results"] = res
    return out
